# revision 22
# baseline (speedup 1.0000x reference)
"""Trainium2 Bass kernel for the Exprnn-style model (nn_Exprnn_2542620639651).

v6: 2-pass linear-scan decomposition, TB=11 timesteps/block (48 blocks),
carry contraction folded into the main matmuls, software-pipelined loop.

Per block (row layout: timestep j -> rows 10j (j<=8) / 106+10(j-9) (j=9,10),
carry rows 96..105, dead rows 90..95):
  ps1 = A1ext @ [u; car1]      2 matmuls (512-col PSUM banks), K=M=126
  s1r = copy(ps1)              ACT evict fp16 (z~' + new car1)
  tt  = clip(s1r, +-|mb|)      DVE tensor_scalar 4x (carry rows pass via 1e30)
  v   = tt + u                 DVE tensor_tensor 2x  (u carry rows are 0)
  car2 insert: v[96:106] <- s2 evict of prev block   (DVE fp16 copy)
  ps2 = A2ext @ [v; car2]      2 matmuls
  s2  = copy(ps2)              ACT [0:SP) + DVE [SP:) evict fp16
  car1 insert: next u tile[96:106] <- s1r[96:106]    (DVE fp16 copy)

The loop is rotated: iteration i emits scan2 of block i-1 BEFORE scan1 of
block i, so the PE never waits on the in-block evict->clip->add chain.
sigma (modrelu sign) is folded into A1ext's output columns; the decoder
Dm=W3@W4 into A2ext's.  Everything on device is fp16; PSUM fp32.
"""

import os
import sys
from contextlib import ExitStack

for _p in ("/root/.axon_site/_ro/trn_rl_repo", "/opt/trn_rl_repo"):
    if os.path.isdir(_p) and _p not in sys.path:
        sys.path.append(_p)

import numpy as np
import ml_dtypes

import concourse.bass as bass
import concourse.tile as tile
from concourse import bacc, mybir
from concourse.bass_utils import run_bass_kernel_spmd

dt = mybir.dt
Alu = mybir.AluOpType
Act = mybir.ActivationFunctionType

B, T, NI, H = 8192, 512, 2, 10
NCORES = 8
NB = B // NCORES          # 1024
TB = 11
NBLK = 48                 # 48*11 = 528 >= 512 (tail padded with zeros)
TPAD = TB * NBLK
KP = 10 * TB              # 110 payload rows
CO = 96                   # carry rows 96..105
M = 126                   # tile rows: 0..89 + dead 90..95 + carry + 106..125
NH = NB // 2              # 512
G = 3                     # blocks per DMA group
NG = NBLK // G            # 16
SP = 640                  # s2 evict split: ACT [0:SP), DVE [SP:NB)
PAYROWS = np.r_[0:90, 106:126]

_cache = {}


def _build_program():
    nc = bacc.Bacc("TRN2", target_bir_lowering=False, debug=False)
    f32, f16 = dt.float32, dt.float16
    uin = nc.dram_tensor("uin", [NG, M, G * NB], f16, kind="ExternalInput").ap()
    da1 = nc.dram_tensor("a1", [M, M], f16, kind="ExternalInput").ap()
    da2 = nc.dram_tensor("a2", [M, M], f16, kind="ExternalInput").ap()
    dchi = nc.dram_tensor("chi", [M, 1], f32, kind="ExternalInput").ap()
    dclo = nc.dram_tensor("clo", [M, 1], f32, kind="ExternalInput").ap()
    yout = nc.dram_tensor("yout", [NG, M, G * NB], f16, kind="ExternalOutput").ap()

    NU, NV, NS, NS1 = 3, 4, 2, 3

    with tile.TileContext(nc) as tc, ExitStack() as ctx:
        wp = ctx.enter_context(tc.tile_pool(name="weights", bufs=1))
        up = ctx.enter_context(tc.tile_pool(name="u", bufs=NU))
        vp = ctx.enter_context(tc.tile_pool(name="v", bufs=NV))
        tp = ctx.enter_context(tc.tile_pool(name="tt", bufs=3))
        s1p = ctx.enter_context(tc.tile_pool(name="s1", bufs=NS1))
        s2p = ctx.enter_context(tc.tile_pool(name="s2", bufs=NS))
        sps = ctx.enter_context(tc.tile_pool(name="ps", bufs=4, space="PSUM"))

        a1 = wp.tile([M, M], f16, tag="a1")
        nc.sync.dma_start(a1[:], da1[:])
        a2 = wp.tile([M, M], f16, tag="a2")
        nc.sync.dma_start(a2[:], da2[:])
        chi = wp.tile([M, 1], f32, tag="chi")
        nc.sync.dma_start(chi[:], dchi[:])
        clo = wp.tile([M, 1], f32, tag="clo")
        nc.sync.dma_start(clo[:], dclo[:])

        uts = []
        for i in range(NU):
            t = up.tile([M, G * NB], f16, tag=f"u{i}")
            uts.append(t)
        vts = []
        for i in range(NV):
            t = vp.tile([M, NB], f16, tag=f"v{i}")
            vts.append(t)
        s2ts = []
        for i in range(NS):
            t = s2p.tile([M, G * NB], f16, tag=f"s2g{i}")
            s2ts.append(t)

        def a1part(b):
            # scan1 of block b + v payload; v's carry rows completed later
            # by copy2 (emitted with a2part(b-1))
            g, bi = divmod(b, G)
            o = bi * NB
            ug = uts[g % NU]
            if bi == 0:
                if g == 0:
                    nc.sync.dma_start(ug[:], uin[g])
                if g + 1 < NG:
                    nc.sync.dma_start(uts[(g + 1) % NU][:], uin[g + 1])
            ps1 = sps.tile([M, NB], f32, tag="ps")
            nc.tensor.matmul(ps1[:, :NH], a1[:], ug[:, o:o + NH],
                             start=True, stop=True)
            nc.tensor.matmul(ps1[:, NH:], a1[:], ug[:, o + NH:o + NB],
                             start=True, stop=True)
            # halved evicts; each half's carry copy for block b+1 emitted
            # immediately so it leads the DVE queue and the carry chain
            # advances per 512-col batch stream
            if b + 1 < NBLK:
                gn, bn = divmod(b + 1, G)
                un, on = uts[gn % NU], bn * NB
            s1r = s1p.tile([M, NB], f16)
            nc.scalar.activation(s1r[:, :NH], ps1[:, :NH], Act.Copy)
            if b + 1 < NBLK:
                nc.vector.tensor_copy(un[CO:CO + 10, on:on + NH],
                                      s1r[CO:CO + 10, :NH])
            nc.scalar.activation(s1r[:, NH:], ps1[:, NH:], Act.Copy)
            if b + 1 < NBLK:
                nc.vector.tensor_copy(un[CO:CO + 10, on + NH:on + NB],
                                      s1r[CO:CO + 10, NH:])
            tt = tp.tile([M, NB], f16)
            nc.vector.tensor_scalar(tt[:], s1r[:], chi[:], clo[:],
                                    Alu.min, Alu.max)
            v = vts[b % NV]
            nc.vector.tensor_add(v[:], tt[:], ug[:, o:o + NB])
            if b == 0:
                nc.vector.memset(v[CO:CO + 10, :], 0.0)

        def a2part(b):
            # scan2 of block b; afterwards emit copy2 completing v(b+1)
            g, bi = divmod(b, G)
            o = bi * NB
            v = vts[b % NV]
            s2g = s2ts[g % NS]
            ps2 = sps.tile([M, NB], f32, tag="ps")
            nc.tensor.matmul(ps2[:, :NH], a2[:], v[:, :NH],
                             start=True, stop=True)
            nc.tensor.matmul(ps2[:, NH:], a2[:], v[:, NH:],
                             start=True, stop=True)
            nc.scalar.activation(s2g[:, o:o + SP], ps2[:, :SP], Act.Copy)
            nc.vector.tensor_copy(s2g[:, o + SP:o + NB], ps2[:, SP:])
            if bi == G - 1:
                nc.sync.dma_start(yout[g], s2g[:])
            if b + 1 < NBLK:
                nc.vector.tensor_copy(vts[(b + 1) % NV][CO:CO + 10, :],
                                      s2g[CO:CO + 10, o:o + NB])

        for i in range(NBLK + 2):
            if i < NBLK:
                a1part(i)
            if i >= 2:
                a2part(i - 2)

    nc.compile()
    return nc


def _prep_inputs(inputs):
    X = np.ascontiguousarray(inputs["X"], dtype=np.float32)
    W1, b1v, W2, b2v = (np.asarray(inputs[k], np.float64) for k in ("W1", "b1", "W2", "b2"))
    Win, R, mbv = (np.asarray(inputs[k], np.float64) for k in ("Win", "R", "mb"))
    W3, b3v, W4, b4v = (np.asarray(inputs[k], np.float64) for k in ("W3", "b3", "W4", "b4"))
    Dm = W3 @ W4
    c4 = (b3v @ W4 + b4v).astype(np.float32)

    Rp = [np.eye(H)]
    for _ in range(TB + 1):
        Rp.append(Rp[-1] @ R)

    sig = np.where(mbv <= 0, -1.0, 1.0)
    Sg = np.diag(sig)
    absmb = np.abs(mbv)

    def rowOf(j):
        return 10 * j if j < 9 else 106 + 10 * (j - 9)

    def lhsExt(dec):
        L = np.zeros((M, M), np.float64)
        for k in range(TB):
            rk = rowOf(k)
            for j in range(k, TB):
                blk = (Rp[j - k] @ Dm) if dec else (Rp[j - k] @ Sg)
                L[rk:rk + 10, rowOf(j):rowOf(j) + 10] = blk
            L[rk:rk + 10, CO:CO + 10] = Rp[TB - 1 - k]
        for j in range(TB):
            blk = (Rp[j + 1] @ Dm) if dec else (Rp[j + 1] @ Sg)
            L[CO:CO + 10, rowOf(j):rowOf(j) + 10] = blk
        L[CO:CO + 10, CO:CO + 10] = Rp[TB]
        return L

    # host encoder + input kernel (<1 GFLOP)
    x1 = np.maximum(X @ W1.astype(np.float32) + b1v.astype(np.float32), 0)
    x2 = np.maximum(x1 @ W2.astype(np.float32) + b2v.astype(np.float32), 0)
    u = x2 @ Win.astype(np.float32)
    Uc = u.reshape(NCORES, NB, T, H)
    Up = np.zeros((NCORES, NB, TPAD, H), np.float32)
    Up[:, :, :T] = Uc
    # row-permuted per-block layout [NCORES, NG, M, G*NB]
    Ul = np.zeros((NCORES, NB, NBLK, M), np.float32)
    Ul[:, :, :, PAYROWS] = Up.reshape(NCORES, NB, NBLK, KP)
    Uin = np.ascontiguousarray(
        Ul.reshape(NCORES, NB, NG, G, M).transpose(0, 2, 4, 3, 1)
        .reshape(NCORES, NG, M, G * NB).astype(np.float16)
    )

    chiv = np.full(M, 1e30, np.float32)
    chiv[PAYROWS] = np.tile(absmb, TB).astype(np.float32)
    shared = {
        "a1": lhsExt(False).astype(np.float16),
        "a2": lhsExt(True).astype(np.float16),
        "chi": np.ascontiguousarray(chiv.reshape(M, 1)),
        "clo": np.ascontiguousarray((-chiv).reshape(M, 1)),
    }
    in_maps = [dict(shared, uin=Uin[c]) for c in range(NCORES)]
    return in_maps, c4


def _gather(results, c4):
    out = np.empty((B, T, H), np.float32)
    for c in range(NCORES):
        yo = np.asarray(results[c]["yout"], dtype=np.float32)  # [NG, M, G*NB]
        pay = yo[:, PAYROWS, :]                                # [NG, KP, G*NB]
        full = (pay.reshape(NG, TB, H, G, NB).transpose(4, 0, 3, 1, 2)
                .reshape(NB, TPAD, H))
        out[c * NB:(c + 1) * NB] = full[:, :T]
    if np.any(c4):
        out += c4
    return out


def kernel(**inputs):
    if "nc" not in _cache:
        _cache["nc"] = _build_program()
    in_maps, c4 = _prep_inputs(inputs)
    res = run_bass_kernel_spmd(_cache["nc"], in_maps, core_ids=list(range(NCORES)))
    return _gather(res.results, c4)
